# revision 1
# baseline (speedup 1.0000x reference)
"""Trainium2 Bass kernel for nn_LCNNConvolution (GNN message passing).

Math:  out[n] = sum_p softplus( gather(X, NS[n,p,:]).flat @ W.T + b ) - 12*ln2
Key transform: W is block-structured over the 8 neighbor slots, so
    x1[n,p,:] = sum_k Y_k[NS[n,p,k]]        with  Y_k = X @ W_k.T
We precompute Y on-chip (PE matmul, fp16) and write it to DRAM as two
bank tables (int16 gather indices only reach 32767 rows), then the hot loop
is an indirect-DMA gather of 128B slot rows + DVE tree-reduction over the 8
slots + ACT softplus + DVE reduction over the 12 perms.

Tricks:
- The gather element is 64 fp16 = 128B (one slot of one site). bass's
  dma_gather asserts elem_size_bytes % 256 == 0, but that restriction only
  exists for transpose mode; the non-transpose Q7 descriptor generator
  handles any element size. We build InstDMAGatherAnt directly, halving
  gather HBM traffic vs the fp32 table.
- Each output row gathers exactly 8 dummy elements (one per slot, from the
  bank its site is NOT in), so filling the dummy rows with b/8 adds exactly
  the Linear bias for free.
- The Y table is split into separate bank-A/bank-B DRAM tensors, and phase 1
  writes the 2.9x-smaller bank B first: bank-B gather calls (issued D_AHEAD
  chunks ahead of bank-A) start ~90us in, overlapping the bank-A writes, so
  the Pool engine's gather stream runs with zero stalls.
- Column-major site layout (site = sitecol*128 + partition) concentrates the
  150 pad sites per core into the last site-column, whose chunk is skipped.
- num_idxs per gather call is capped at 1024 (empirical HW limit; 1280
  crashes the device - boundary-probed).

Sharding: data-parallel over sites; each of the 8 cores handles 6250 sites
and computes its own full Y copy (replicated X / W).
"""

import math
import os

import numpy as np

import concourse.bass as bass
import concourse.bacc as bacc
import concourse.mybir as mybir
import concourse.tile as tile
from concourse import ap_utils
from concourse.bass import MemorySpace, exact_div
from concourse.bass_utils import run_bass_kernel_spmd

# ---------------------------------------------------------------- constants
N_SITES = 50000
NODE_F = 64
N_PERM = 12
N_NEIGH = 8
OUT_F = 64
LN2 = float(np.log(2.0))

N_CORES = 8
SITES_PER_CORE = N_SITES // N_CORES            # 6250
SITES_PER_PART = 50                            # ceil(6250/128) padded to 50
PAD_SITES = 128 * SITES_PER_PART               # 6400
COLS = SITES_PER_PART * N_PERM                 # 600 rows (n,p) per partition
GCOLS = 8                                      # cols per dma_gather call
N_CHUNKS = COLS // GCOLS                       # 75 chunk grid (last = padding)
N_CHUNKS_EFF = N_CHUNKS - 1                    # chunks actually processed
NIDX = 128 * GCOLS                             # 1024 gathers/call (HW limit)
IWRAP = NIDX // 16                             # idx cols per 16-partition wrap
def set_gcols(g):
    """Reconfigure the gather call size (for HW-limit experiments)."""
    global GCOLS, N_CHUNKS, NIDX, IWRAP
    assert COLS % g == 0 and 24 % g == 0
    GCOLS, N_CHUNKS, NIDX, IWRAP = g, COLS // g, 128 * g, 128 * g // 16


BANK = 32767                                   # bank A covers sites [0, 32767)
DUMMY_B = 50001 - BANK                         # = 17234, Z2 row of bank B
NROWS_B = DUMMY_B + 1                          # bank B table rows
D_AHEAD = 20                                   # bank-B chunks issued ahead

XT_HALF = 25088                                # 196*128, top half site count

F32 = mybir.dt.float32
F16 = mybir.dt.float16
I16 = mybir.dt.int16


def dma_gather_128(nc, out_ap, in_ap, idxs_ap, num_idxs, elem_size, elem_step,
                   queue_num=0):
    """Non-transpose DRAM-source dma_gather without the 256B-element floor.

    Mirrors BassGpSimd.dma_gather for the (transpose=False, DRAM source,
    prepare_only=False) case; elem_size is in table-dtype elements.
    """
    gp = nc.gpsimd
    assert idxs_ap.dtype == mybir.dt.int16
    assert in_ap.dtype == out_ap.dtype
    assert in_ap.space == MemorySpace.DRAM
    assert idxs_ap.space == MemorySpace.SBUF
    assert out_ap.space == MemorySpace.SBUF
    assert ap_utils.ap_is_contiguous(out_ap.ap[1:])
    assert ap_utils.ap_is_contiguous(idxs_ap.ap[1:])
    assert in_ap.ap[-1][1] == out_ap.ap[-1][1] == elem_size
    assert out_ap.ap[0][1] * out_ap.ap[1][1] == num_idxs
    assert in_ap.ap[0][0] == elem_step
    stride_bytes = elem_step * mybir.dt.size(in_ap.dtype)
    stride_bytes_256 = exact_div(stride_bytes, 256)
    assert stride_bytes_256 < 256

    _in_ap = gp.lower_ap_dma(in_ap, for_custom_bir_dma=True)
    _idxs_ap = gp.lower_ap(idxs_ap)
    _out_ap = gp.lower_ap(out_ap)
    return gp.add_instruction(
        mybir.InstDMAGatherAnt(
            name=nc.get_next_instruction_name(),
            ins=[
                *_in_ap,
                _idxs_ap,
                gp.lower_val_access(gp.to_reg(num_idxs)),
            ],
            outs=[_out_ap],
            transpose=False,
            num_idxs=num_idxs,
            elem_size=elem_size,
            stride_bytes_256=stride_bytes_256,
            gen_mode=0,
            single_packet=True,
            queue_num=queue_num,
            sbuf_tokens_per_rank=0,
            sbuf_free_dim_per_rank=0,
            sbuf_free_dim_pad_per_rank=0,
            sbuf_byte_offset=0,
        )
    )


# ---------------------------------------------------------------- device IR
def build_nc(scratch=16384):
    nc = bacc.Bacc(
        "TRN2",
        target_bir_lowering=False,
        debug=False,
        dynamic_dma_scratch_size=scratch,
        num_swdge_queues=4,
    )

    xt = nc.dram_tensor("xt", [128, XT_HALF], F16, kind="ExternalInput").ap()
    wt = nc.dram_tensor("wt", [128, 512], F16, kind="ExternalInput").ap()
    # "zero" row content: b/8 tiled over the 8 slot blocks (see module doc)
    bz = nc.dram_tensor("bz", [1, 512], F16, kind="ExternalInput").ap()
    # per chunk: 16 (bank, slot) index sets, 16-partition-wrapped and
    # host-replicated to the 8 16-partition groups the Q7 cores read (a plain
    # full-width DMA load keeps the gather prologue off the busy DVE)
    idx = nc.dram_tensor(
        "idx", [N_CHUNKS_EFF, 128, 16 * IWRAP], I16, kind="ExternalInput"
    ).ap()
    out = nc.dram_tensor(
        "out", [128, SITES_PER_PART, OUT_F], F32, kind="ExternalOutput"
    ).ap()

    with tile.TileContext(nc) as tc:
        with (
            tc.tile_pool(name="persist", bufs=1) as persist,
            tc.tile_pool(name="dramA", bufs=1, space="DRAM") as dramA,
            tc.tile_pool(name="dramB", bufs=1, space="DRAM") as dramB,
        ):
            half_sb = persist.tile([128, 1], F32)
            nc.vector.memset(half_sb[:], 0.5)

            # bank A: row 0 = Z (b/8), rows 1+s = sites 0..32766
            # bank B: row 0 unused, rows s-32766 = sites 32767..49999,
            #         row DUMMY_B = Z2 (b/8)
            tabA = dramA.tile([BANK + 1, 512], F16)
            tabB = dramB.tile([NROWS_B, 512], F16)
            zrow = persist.tile([1, 512], F16)
            nc.sync.dma_start(out=zrow[:], in_=bz[:])
            nc.sync.dma_start(out=tabA[0:1, :], in_=zrow[:])
            nc.sync.dma_start(out=tabB[DUMMY_B : DUMMY_B + 1, :], in_=zrow[:])
            # tabB row 0 is an addressing pad (never indexed); initialize it
            # so finiteness checks on the gather's table view stay clean
            nc.sync.dma_start(out=tabB[0:1, :], in_=zrow[:])

            def y_row(s):  # (table, row) of site s
                if s <= BANK - 1:
                    return tabA, 1 + s
                return tabB, s - (BANK - 1)

            # ---------------- phase 1: Y = X @ Wall.T
            # all pools (both phases) share one scope: phase-2 tiles must
            # NOT reuse phase-1 SBUF, or their first writes pick up WAR
            # dependencies on phase-1 reads and the phases serialize
            with (
                tc.tile_pool(name="p1", bufs=1) as p1,
                tc.tile_pool(name="p1y", bufs=3) as p1y,
                tc.tile_pool(name="p1ps", bufs=8, space="PSUM") as p1ps,
                tc.tile_pool(name="p2g", bufs=3) as p2g,
                tc.tile_pool(name="p2i", bufs=D_AHEAD + 2) as p2i,
                tc.tile_pool(name="p2t", bufs=2) as p2t,
                tc.tile_pool(name="p2p", bufs=D_AHEAD + 2) as p2p,
                tc.tile_pool(name="p2o", bufs=2) as p2o,
            ):
                xt_sb = p1.tile([128, XT_HALF], F16)
                # h=1 half first: the bank-B site blocks written first only
                # read xt rows 64:128
                nc.sync.dma_start(out=xt_sb[64:128, :], in_=xt[64:128, :])
                nc.sync.dma_start(out=xt_sb[0:64, :], in_=xt[0:64, :])
                wt_sb = p1.tile([128, 512], F16)
                nc.sync.dma_start(out=wt_sb[:], in_=wt[:])

                # group GRP 128-site blocks into one Y-write DMA each (one
                # HWDGE slot per GRP blocks instead of per block); a group
                # must be uniform: full 128-row blocks, site-contiguous, and
                # entirely within one bank table
                GRP = 4
                blocks = []
                for h in range(2):
                    for j in range(XT_HALF // 128):
                        s0 = h * XT_HALF + j * 128
                        if s0 >= N_SITES:
                            break
                        blocks.append((h, j, s0, min(128, N_SITES - s0)))
                # write bank B (2.9x smaller) first: its gather calls can
                # then start while bank A's rows are still being written
                blocks.sort(key=lambda b: (b[2] + b[3] <= BANK, b[2]))
                blk = 0
                gi = 0
                while gi < len(blocks):
                    grp = blocks[gi : gi + GRP]
                    uniform = (
                        len(grp) == GRP
                        and all(b[3] == 128 for b in grp)
                        and all(
                            grp[i + 1][2] == grp[i][2] + 128
                            for i in range(len(grp) - 1)
                        )
                        and y_row(grp[0][2])[0] is y_row(grp[-1][2] + 127)[0]
                    )
                    if uniform:
                        y_grp = p1y.tile(
                            [128, GRP, 512], F16, tag="y", name="y_grp"
                        )
                    else:
                        y_grp = None
                    for bi, (h, j, s0, nrows) in enumerate(grp):
                        psum = p1ps.tile([128, 512], F32, space="PSUM", tag="ps")
                        lhsT = xt_sb[64 * h : 64 * h + 64, j * 128 : (j + 1) * 128]
                        nc.tensor.matmul(
                            out=psum[:, 0:512],
                            lhsT=lhsT,
                            rhs=wt_sb[64 * h : 64 * h + 64, 0:512],
                            start=True,
                            stop=True,
                        )
                        dst = (
                            y_grp[:, bi, :]
                            if uniform
                            else p1y.tile([128, 512], F16, tag="yt", name="y_tail")
                        )
                        # alternate the PSUM->SBUF cast between ACT and DVE
                        if blk % 2 == 0:
                            nc.scalar.copy(out=dst[:], in_=psum[:])
                        else:
                            nc.vector.tensor_copy(out=dst[:], in_=psum[:])
                        blk += 1
                        if not uniform:
                            # split the block's rows across the bank tables
                            r = 0
                            while r < nrows:
                                t, row = y_row(s0 + r)
                                run = nrows - r
                                if t is tabA:
                                    run = min(run, (BANK - 1) - (s0 + r) + 1)
                                nc.sync.dma_start(
                                    out=t[row : row + run, :],
                                    in_=dst[r : r + run, :],
                                )
                                r += run
                    if uniform:
                        t, row = y_row(grp[0][2])
                        nc.sync.dma_start(
                            out=t[row : row + GRP * 128, :].rearrange(
                                "(b r) f -> r b f", b=GRP
                            ),
                            in_=y_grp[:],
                        )
                    gi += len(grp)

                # ---------------- phase 2: gathers (2 banks x 8 slots) + reduce
                    # Group kb = bank*8 + slot. For output row r, slot k: site s
                    # gathered from its bank table (idx per y_row(); dummy -> b/8
                    # row of the other bank). x1 = sum of all 16 groups. Bank-B
                    # calls for chunk j+D_AHEAD are issued before bank-A calls
                    # for chunk j; tab B is written first in phase 1, so the
                    # prologue's B gathers overlap the bank-A table writes.
                RC = 24  # softplus/perm-reduce group: 2 sites
                CPG = RC // GCOLS
                idx_tiles = {}
                p0_tiles = {}

                def load_idx(j):
                    idx_sb = p2i.tile([128, 16 * IWRAP], I16, tag="idx")
                    nc.sync.dma_start(out=idx_sb[:], in_=idx[j])
                    idx_tiles[j] = idx_sb

                def bank_calls(j, bank):
                    idx_sb = idx_tiles[j]
                    g = p2g.tile([128, 8, GCOLS, 64], F16, tag="g")
                    for k in range(8):
                        kb = bank * 8 + k
                        if bank == 0:
                            tab = tabA[:, 64 * k : 64 * (k + 1)]
                        else:
                            tab = tabB[:, 64 * k : 64 * (k + 1)]
                        dma_gather_128(
                            nc,
                            out_ap=g[:, k, :, :],
                            in_ap=tab,
                            idxs_ap=idx_sb[:, kb * IWRAP : (kb + 1) * IWRAP],
                            num_idxs=NIDX,
                            elem_size=64,
                            elem_step=512,
                            queue_num=kb % 4,
                        )
                    # tree-reduce the 8 slots of this bank (unit-stride fp16
                    # adds keep the DVE 2x mode)
                    t4 = p2t.tile([128, 4, GCOLS, OUT_F], F16, tag="t4")
                    nc.vector.tensor_tensor(
                        out=t4[:], in0=g[:, 0:4], in1=g[:, 4:8],
                        op=mybir.AluOpType.add,
                    )
                    nc.vector.tensor_tensor(
                        out=t4[:, 0:2], in0=t4[:, 0:2], in1=t4[:, 2:4],
                        op=mybir.AluOpType.add,
                    )
                    p_b = p2p.tile([128, GCOLS, OUT_F], F16, tag="p0")
                    nc.vector.tensor_tensor(
                        out=p_b[:], in0=t4[:, 0], in1=t4[:, 1],
                        op=mybir.AluOpType.add,
                    )
                    return p_b

                # prologue: bank-B gathers for the first D_AHEAD chunks
                # (tab B is written first in phase 1, so these overlap the
                # bank-A table writes)
                for j in range(min(D_AHEAD, N_CHUNKS_EFF)):
                    load_idx(j)
                    p0_tiles[j] = bank_calls(j, 1)

                x1 = None
                for j in range(N_CHUNKS_EFF):
                    sub = j % CPG
                    if sub == 0:
                        x1 = p2o.tile([128, RC, OUT_F], F16, tag="x1")
                    c0, c1 = sub * GCOLS, (sub + 1) * GCOLS
                    p_b = bank_calls(j, 0)
                    if j + D_AHEAD < N_CHUNKS_EFF:
                        load_idx(j + D_AHEAD)
                        p0_tiles[j + D_AHEAD] = bank_calls(j + D_AHEAD, 1)
                    nc.vector.tensor_tensor(
                        out=x1[:, c0:c1], in0=p_b[:], in1=p0_tiles.pop(j)[:],
                        op=mybir.AluOpType.add,
                    )
                    del idx_tiles[j]
                    last = j == N_CHUNKS_EFF - 1
                    if sub != CPG - 1 and not last:
                        continue
                    grp = j // CPG
                    nsite = 1 if last else RC // N_PERM
                    nc_cols = nsite * N_PERM
                    # softplus(x) - ln2 == Ln(0.5*Exp(x) + 0.5)
                    x2 = p2o.tile([128, RC, OUT_F], F32, tag="x2")
                    nc.scalar.activation(
                        out=x2[:, 0:nc_cols],
                        in_=x1[:, 0:nc_cols],
                        func=mybir.ActivationFunctionType.Exp,
                    )
                    nc.scalar.activation(
                        out=x2[:, 0:nc_cols],
                        in_=x2[:, 0:nc_cols],
                        func=mybir.ActivationFunctionType.Ln,
                        scale=0.5,
                        bias=half_sb[:],
                    )
                    # out[p, s, f] = sum_q x2[p, s*12+q, f]
                    acc = p2o.tile([128, RC // N_PERM, OUT_F], F32, tag="acc")
                    nc.vector.tensor_reduce(
                        out=acc[:, 0:nsite],
                        in_=x2[:, 0:nc_cols].rearrange(
                            "p (s q) f -> p s f q", q=N_PERM
                        ),
                        axis=mybir.AxisListType.X,
                        op=mybir.AluOpType.add,
                    )
                    nc.sync.dma_start(
                        out=out[:, grp * 2 : grp * 2 + nsite, :],
                        in_=acc[:, 0:nsite],
                    )

    nc.compile()
    return nc


# ---------------------------------------------------------------- host side
def _host_prep(X_sites, X_NSs, W, b):
    X_sites = np.asarray(X_sites, dtype=np.float32)
    X_NSs = np.asarray(X_NSs)
    W = np.asarray(W, dtype=np.float32)
    b = np.asarray(b, dtype=np.float32)

    xt = np.zeros((128, XT_HALF), dtype=np.float32)
    xt[:64, :] = X_sites[:XT_HALF].T
    xt[64:, : N_SITES - XT_HALF] = X_sites[XT_HALF:].T
    xt = xt.astype(np.float16)

    wt = np.ascontiguousarray(
        np.tile(
            W.reshape(OUT_F, N_NEIGH, NODE_F).transpose(2, 1, 0).reshape(NODE_F, 512),
            (2, 1),
        )
    ).astype(np.float16)
    bz = np.ascontiguousarray(np.tile(b / 8.0, N_NEIGH).reshape(1, 512)).astype(
        np.float16
    )

    in_maps = []
    for c in range(N_CORES):
        sl = X_NSs[c * SITES_PER_CORE : (c + 1) * SITES_PER_CORE]
        sl = np.concatenate(
            [sl, np.zeros((PAD_SITES - SITES_PER_CORE, N_PERM, N_NEIGH), sl.dtype)]
        )
        s = (
            sl.reshape(SITES_PER_PART, 128, N_PERM, N_NEIGH)
            .transpose(1, 0, 2, 3)
            .astype(np.int64)
        )
        # bank A: idx = s+1 (row 1+s), dummy 0 -> Z row
        a = np.where(s <= BANK - 1, s + 1, 0)
        # bank B: idx = s-(BANK-1), dummy DUMMY_B -> Z2 row
        bk = np.where(s >= BANK, s - (BANK - 1), DUMMY_B)
        # V[p, cols, kb] with kb = bank*8 + k, cols = site_col*12 + q
        V = np.concatenate([a, bk], axis=-1).reshape(128, COLS, 16)
        # per call (chunk, kb): position i = c*128 + p over GCOLS cols
        arr = V.reshape(128, N_CHUNKS, GCOLS, 16).transpose(1, 3, 2, 0)
        arr = arr.reshape(N_CHUNKS, 16, NIDX)
        # 16-partition wrap: tile[p_row, col] = arr[col*16 + p_row]
        t16 = arr.reshape(N_CHUNKS, 16, IWRAP, 16).transpose(0, 1, 3, 2)
        full = np.tile(
            t16.transpose(0, 2, 1, 3).reshape(N_CHUNKS, 16, NIDX), (1, 8, 1)
        ).astype(np.int16)[:N_CHUNKS_EFF]
        in_maps.append({"xt": xt, "wt": wt, "bz": bz, "idx": full})
    return in_maps


_NC_CACHE = {}


def _get_nc():
    if "nc" not in _NC_CACHE:
        _NC_CACHE["nc"] = build_nc()
    return _NC_CACHE["nc"]


def _stitch(results):
    full = np.empty((N_SITES, OUT_F), dtype=np.float32)
    for c, r in enumerate(results):
        o = (
            r["out"].reshape(128, SITES_PER_PART, OUT_F)
            .transpose(1, 0, 2)
            .reshape(PAD_SITES, OUT_F)[:SITES_PER_CORE]
        )
        full[c * SITES_PER_CORE : (c + 1) * SITES_PER_CORE] = o
    return full


def kernel(X_sites, X_NSs, W, b, _trace=False):
    nc = _get_nc()
    in_maps = _host_prep(X_sites, X_NSs, W, b)
    res = run_bass_kernel_spmd(
        nc, in_maps, core_ids=list(range(N_CORES)), trace=_trace
    )
    full = _stitch(res.results)
    if _trace:
        return full, res
    return full



# revision 11
# speedup vs baseline: 1.1181x; 1.1181x over previous
"""Trainium2 Bass kernel for nn_LCNNConvolution (GNN message passing).

Math:  out[n] = sum_p softplus( gather(X, NS[n,p,:]).flat @ W.T + b ) - 12*ln2
Key transform: W is block-structured over the 8 neighbor slots, so
    x1[n,p,:] = sum_k Y_k[NS[n,p,k]]        with  Y_k = X @ W_k.T + b/8
We precompute Y on-chip (PE matmul, fp16; the +b/8 comes from an extra K=1
matmul with a ones-row against b/8 accumulated into the same PSUM) and write
it to DRAM as two bank tables (int16 gather indices only reach 32767 rows).

v2 (sorted-pattern scheduling): the Pool engine's SWDGE descriptor generation
(994ns fixed + 0.34ns/idx per call, 1024-idx call cap) is the bottleneck, and
the naive dual-bank scheme costs 16 calls per 1024-position chunk (every slot
needs both banks, with dummy zero-row fetches covering the other bank's
positions).  Instead, positions are sorted by the Gray code of their 8-bit
bank pattern, so each 1024-chunk is single-bank in most slots: ~10.3 calls
per chunk (union across the 8 cores; the per-chunk call structure is computed
at runtime from the actual indices and baked into the compiled module).
x1 = sum of all call tiles (dummy rows fetch zeros; bias lives in Y itself).

Sorting scrambles the 12 perms of each site, so the final perm-reduction is
done by writing softplus results to a DRAM staging table (fp32, 256B rows,
in sorted order) and gathering them back per (perm, site-block).  To keep
the gather-back single-bank (int16 again), positions are pre-partitioned
into 3 perm-groups (perms 0-3 / 4-7 / 8-11), each with <=25600 staging rows.

Phase-1/phase-2 overlap: bank-B (2.9x smaller) is written first, and each
chunk's bank-B calls + B-tree run D_AHEAD chunks ahead of its bank-A calls,
so the gather stream starts while bank-A rows are still being written.

Sharding: data-parallel over sites; each of the 8 cores handles 6250 sites
and computes its own full Y copy (replicated X / W).
"""

import functools

import numpy as np

import concourse.bass as bass
import concourse.bacc as bacc
import concourse.mybir as mybir
import concourse.tile as tile
from concourse import ap_utils, hw_specs
from concourse.bass import MemorySpace, exact_div
from concourse.bass_utils import run_bass_kernel_spmd

# Keep Exp and Ln in one activation-table set: by default the table chooser
# may alternate between per-function sets, inserting a 1283ns
# InstLoadActFuncSet before every softplus half.  Hide exp/ln from all sets
# except the combined one (list positions are preserved, so the emitted
# act_func_set_id still indexes act_info.json correctly).
_orig_get_act_tables = hw_specs.get_activation_tables


@functools.cache
def _patched_act_tables(module_arch):
    tabs = dict(_orig_get_act_tables(module_arch))
    exp_ln = {
        mybir.ActivationFunctionType.Exp,
        mybir.ActivationFunctionType.Ln,
    }
    combined = [n for n, s in tabs.items() if exp_ln <= s]
    if combined:
        keep = combined[0]
        tabs = {
            n: (s if n == keep else s - exp_ln) for n, s in tabs.items()
        }
    return tabs


hw_specs.get_activation_tables = _patched_act_tables
bacc.get_activation_tables = _patched_act_tables

# ---------------------------------------------------------------- constants
N_SITES = 50000
NODE_F = 64
N_PERM = 12
N_NEIGH = 8
OUT_F = 64
LN2 = float(np.log(2.0))

N_CORES = 8
SITES_PER_CORE = N_SITES // N_CORES            # 6250

BANK = 32767          # bank A holds sites 0..32766 at rows 1..32767; row 0 = 0
NROWS_B = 17234       # bank B rows: 0 = zero row, 1..17233 = sites 32767..49999

N_GROUPS = 3          # perm-groups 0-3 / 4-7 / 8-11
PERMS_PER_G = 4
POS_PER_G = SITES_PER_CORE * PERMS_PER_G       # 25000
CH_PER_G = 25                                  # 1024-position chunks per group
CAP_PER_G = CH_PER_G * 1024                    # 25600 staging rows
NIDX = 1024
IWRAP = NIDX // 16                             # 64 idx cols per 16-part group

U_CALLS_PER_PERM = 7                           # 7 x 1024 site-entries (6250+pad)
U_SITES = U_CALLS_PER_PERM * 1024              # 7168 (rows 6250+ are pad)

XT_HALF = 25088                                # 196*128, top half site count
D_AHEAD = 12                                   # bank-B streams ahead (chunks)
L_AHEAD = 2                                    # idx loads ahead of the B stream

F32 = mybir.dt.float32
F16 = mybir.dt.float16
I16 = mybir.dt.int16


def dma_gather_128(nc, out_ap, in_ap, idxs_ap, num_idxs, elem_size, elem_step,
                   queue_num=0):
    """Non-transpose DRAM-source dma_gather without the 256B-element floor.

    Mirrors BassGpSimd.dma_gather for the (transpose=False, DRAM source,
    prepare_only=False) case; elem_size is in table-dtype elements.
    """
    gp = nc.gpsimd
    assert idxs_ap.dtype == mybir.dt.int16
    assert in_ap.dtype == out_ap.dtype
    assert in_ap.space == MemorySpace.DRAM
    assert idxs_ap.space == MemorySpace.SBUF
    assert out_ap.space == MemorySpace.SBUF
    assert ap_utils.ap_is_contiguous(out_ap.ap[1:])
    assert ap_utils.ap_is_contiguous(idxs_ap.ap[1:])
    assert in_ap.ap[-1][1] == out_ap.ap[-1][1] == elem_size
    assert out_ap.ap[0][1] * out_ap.ap[1][1] == num_idxs
    assert in_ap.ap[0][0] == elem_step
    stride_bytes = elem_step * mybir.dt.size(in_ap.dtype)
    stride_bytes_256 = exact_div(stride_bytes, 256)
    assert stride_bytes_256 < 256

    _in_ap = gp.lower_ap_dma(in_ap, for_custom_bir_dma=True)
    _idxs_ap = gp.lower_ap(idxs_ap)
    _out_ap = gp.lower_ap(out_ap)
    return gp.add_instruction(
        mybir.InstDMAGatherAnt(
            name=nc.get_next_instruction_name(),
            ins=[
                *_in_ap,
                _idxs_ap,
                gp.lower_val_access(gp.to_reg(num_idxs)),
            ],
            outs=[_out_ap],
            transpose=False,
            num_idxs=num_idxs,
            elem_size=elem_size,
            stride_bytes_256=stride_bytes_256,
            gen_mode=0,
            single_packet=True,
            queue_num=queue_num,
            sbuf_tokens_per_rank=0,
            sbuf_free_dim_per_rank=0,
            sbuf_free_dim_pad_per_rank=0,
            sbuf_byte_offset=0,
        )
    )


# ---------------------------------------------------------------- device IR
def build_nc(structure):
    """structure: per (group, chunk): (tuple of bank-A slots, tuple of bank-B
    slots) that need a gather call.  Call/idx issue order must match
    _host_prep: per chunk, B-calls first (B-slot order), then A-calls."""
    nc = bacc.Bacc(
        "TRN2",
        target_bir_lowering=False,
        debug=False,
        dynamic_dma_scratch_size=16384,
        num_swdge_queues=4,
    )

    total_calls = sum(len(a) + len(b) for g in structure for (a, b) in g)
    max_a = max(len(a) for g in structure for (a, _) in g)
    max_b = max(len(b) for g in structure for (_, b) in g)

    xt = nc.dram_tensor("xt", [128, XT_HALF], F16, kind="ExternalInput").ap()
    wt = nc.dram_tensor("wt", [128, 512], F16, kind="ExternalInput").ap()
    bz = nc.dram_tensor("bz", [128, 512], F16, kind="ExternalInput").ap()
    gidx = nc.dram_tensor(
        "gidx", [128, total_calls * IWRAP], I16, kind="ExternalInput"
    ).ap()
    uidx = nc.dram_tensor(
        "uidx", [N_GROUPS, PERMS_PER_G, U_CALLS_PER_PERM, 128, IWRAP], I16,
        kind="ExternalInput",
    ).ap()
    out = nc.dram_tensor(
        "out", [U_CALLS_PER_PERM, 128, 8, OUT_F], F32, kind="ExternalOutput"
    ).ap()

    with tile.TileContext(nc) as tc:
        with (
            tc.tile_pool(name="persist", bufs=1) as persist,
            tc.tile_pool(name="dramA", bufs=1, space="DRAM") as dramA,
            tc.tile_pool(name="dramB", bufs=1, space="DRAM") as dramB,
            tc.tile_pool(name="dramS", bufs=1, space="DRAM") as dramS,
        ):
            half_sb = persist.tile([128, 1], F32)
            nc.vector.memset(half_sb[:], 0.5)
            zrow = persist.tile([1, 512], F16)
            nc.vector.memset(zrow[:], 0.0)
            # full bias b tiled over the 8 slot blocks, replicated to all
            # partitions on host; added once per chunk to x1 before softplus
            bzb = persist.tile([128, 512], F16)
            nc.sync.dma_start(out=bzb[:], in_=bz[:])

            tabA = dramA.tile([BANK + 1, 512], F16)
            tabB = dramB.tile([NROWS_B, 512], F16)
            stag = [dramS.tile([CAP_PER_G, 64], F32, name=f"st{g}")
                    for g in range(N_GROUPS)]
            nc.sync.dma_start(out=tabA[0:1, :], in_=zrow[:])
            nc.sync.dma_start(out=tabB[0:1, :], in_=zrow[:])

            def y_row(s):  # (table, row) of site s
                if s <= BANK - 1:
                    return tabA, 1 + s
                return tabB, s - (BANK - 1)

            # acc tiles for the final perm-sum live across the whole run
            acc = [persist.tile([128, 8, OUT_F], F32, name=f"acc{j}")
                   for j in range(U_CALLS_PER_PERM)]

            with (
                tc.tile_pool(name="p1", bufs=1) as p1,
                tc.tile_pool(name="p1y", bufs=3) as p1y,
                tc.tile_pool(name="p1ps", bufs=8, space="PSUM") as p1ps,
                tc.tile_pool(name="p2gA", bufs=3) as p2gA,
                tc.tile_pool(name="p2gB", bufs=2) as p2gB,
                tc.tile_pool(name="p2i", bufs=D_AHEAD + L_AHEAD + 3) as p2i,
                tc.tile_pool(name="p2p", bufs=D_AHEAD + 3) as p2p,
                tc.tile_pool(name="p2o", bufs=3) as p2o,
                tc.tile_pool(name="p3i", bufs=12) as p3i,
                tc.tile_pool(name="p3g", bufs=3) as p3g,
            ):
                xt_sb = p1.tile([128, XT_HALF], F16)
                # h=1 half first: the bank-B site blocks written first only
                # read xt rows 64:128
                nc.sync.dma_start(out=xt_sb[64:128, :], in_=xt[64:128, :])
                nc.sync.dma_start(out=xt_sb[0:64, :], in_=xt[0:64, :])
                wt_sb = p1.tile([128, 512], F16)
                nc.sync.dma_start(out=wt_sb[:], in_=wt[:])

                # ---------------- phase 1: Y = X @ Wall.T + b/8 (per slot)
                GRP = 4
                blocks = []
                for h in range(2):
                    for j in range(XT_HALF // 128):
                        s0 = h * XT_HALF + j * 128
                        if s0 >= N_SITES:
                            break
                        blocks.append((h, j, s0, min(128, N_SITES - s0)))
                # bank B (2.9x smaller) first so B gathers can start early
                blocks.sort(key=lambda b: (b[2] + b[3] <= BANK - 1, b[2]))
                blk = 0
                gi = 0
                while gi < len(blocks):
                    grp = blocks[gi : gi + GRP]
                    uniform = (
                        len(grp) == GRP
                        and all(b[3] == 128 for b in grp)
                        and all(
                            grp[i + 1][2] == grp[i][2] + 128
                            for i in range(len(grp) - 1)
                        )
                        and y_row(grp[0][2])[0] is y_row(grp[-1][2] + 127)[0]
                    )
                    if uniform:
                        y_grp = p1y.tile(
                            [128, GRP, 512], F16, tag="y", name="y_grp"
                        )
                    else:
                        y_grp = None
                    for bi, (h, j, s0, nrows) in enumerate(grp):
                        psum = p1ps.tile([128, 512], F32, space="PSUM", tag="ps")
                        lhsT = xt_sb[64 * h : 64 * h + 64, j * 128 : (j + 1) * 128]
                        nc.tensor.matmul(
                            out=psum[:, 0:512],
                            lhsT=lhsT,
                            rhs=wt_sb[64 * h : 64 * h + 64, 0:512],
                            start=True,
                            stop=True,
                        )
                        dst = (
                            y_grp[:, bi, :]
                            if uniform
                            else p1y.tile([128, 512], F16, tag="yt", name="y_tail")
                        )
                        if blk % 2 == 0:
                            nc.scalar.copy(out=dst[:], in_=psum[:])
                        else:
                            nc.vector.tensor_copy(out=dst[:], in_=psum[:])
                        blk += 1
                        if not uniform:
                            r = 0
                            while r < nrows:
                                t, row = y_row(s0 + r)
                                run = nrows - r
                                if t is tabA:
                                    run = min(run, (BANK - 1) - (s0 + r) + 1)
                                nc.sync.dma_start(
                                    out=t[row : row + run, :],
                                    in_=dst[r : r + run, :],
                                )
                                r += run
                    if uniform:
                        t, row = y_row(grp[0][2])
                        nc.sync.dma_start(
                            out=t[row : row + GRP * 128, :].rearrange(
                                "(b r) f -> r b f", b=GRP
                            ),
                            in_=y_grp[:],
                        )
                    gi += len(grp)

                # ---------------- phase 2 + 3, pipelined per group
                # flat chunk list [(g, c, (aslots, bslots), gidx_base), ...]
                chunks = []
                base = 0
                for g in range(N_GROUPS):
                    for c in range(CH_PER_G):
                        a, b = structure[g][c]
                        chunks.append((g, c, a, b, base))
                        base += len(b) + len(a)
                nchunks = len(chunks)

                idx_tiles = {}
                pB_tiles = {}

                def load_idx(ci):
                    g, c, a, b, gb = chunks[ci]
                    nc_calls = len(a) + len(b)
                    t = p2i.tile([128, nc_calls * IWRAP], I16, tag="idx")
                    nc.sync.dma_start(
                        out=t[:],
                        in_=gidx[:, gb * IWRAP : (gb + nc_calls) * IWRAP],
                    )
                    idx_tiles[ci] = t

                def b_calls(ci):
                    """Bank-B gathers for chunk ci + tree-reduce to one tile."""
                    g, c, a, b, gb = chunks[ci]
                    nB = len(b)
                    if nB == 0:
                        pB_tiles[ci] = None
                        return
                    idx_sb = idx_tiles[ci]
                    gt = p2gB.tile([128, max_b, 8, OUT_F], F16, tag="gB")
                    for i, k in enumerate(b):
                        dma_gather_128(
                            nc,
                            out_ap=gt[:, i, :, :],
                            in_ap=tabB[:, 64 * k : 64 * (k + 1)],
                            idxs_ap=idx_sb[:, i * IWRAP : (i + 1) * IWRAP],
                            num_idxs=NIDX,
                            elem_size=64,
                            elem_step=512,
                            queue_num=(ci + i) % 4,
                        )
                    m = nB
                    while m > 1:
                        h = m // 2
                        nc.vector.tensor_tensor(
                            out=gt[:, 0:h], in0=gt[:, 0:h], in1=gt[:, m - h : m],
                            op=mybir.AluOpType.add,
                        )
                        m = m - h
                    pB = p2p.tile([128, 8, OUT_F], F16, tag="pB")
                    nc.vector.tensor_copy(out=pB[:], in_=gt[:, 0])
                    pB_tiles[ci] = pB

                def a_calls_and_finish(ci):
                    g, c, a, b, gb = chunks[ci]
                    nA, nB = len(a), len(b)
                    idx_sb = idx_tiles[ci]
                    gt = None
                    if nA:
                        gt = p2gA.tile([128, max_a, 8, OUT_F], F16, tag="gA")
                        for i, k in enumerate(a):
                            dma_gather_128(
                                nc,
                                out_ap=gt[:, i, :, :],
                                in_ap=tabA[:, 64 * k : 64 * (k + 1)],
                                idxs_ap=idx_sb[
                                    :, (nB + i) * IWRAP : (nB + i + 1) * IWRAP
                                ],
                                num_idxs=NIDX,
                                elem_size=64,
                                elem_step=512,
                                queue_num=(ci + i) % 4,
                            )
                        m = nA
                        while m > 1:
                            h = m // 2
                            nc.vector.tensor_tensor(
                                out=gt[:, 0:h], in0=gt[:, 0:h],
                                in1=gt[:, m - h : m],
                                op=mybir.AluOpType.add,
                            )
                            m = m - h
                    pB = pB_tiles.pop(ci)
                    x1 = p2o.tile([128, 8, OUT_F], F16, tag="x1")
                    bzv = bzb[:].rearrange("p (c f) -> p c f", f=OUT_F)
                    if gt is not None and pB is not None:
                        nc.vector.tensor_tensor(
                            out=x1[:], in0=gt[:, 0], in1=pB[:],
                            op=mybir.AluOpType.add,
                        )
                        nc.vector.tensor_tensor(
                            out=x1[:], in0=x1[:], in1=bzv,
                            op=mybir.AluOpType.add,
                        )
                    elif gt is not None:
                        nc.vector.tensor_tensor(
                            out=x1[:], in0=gt[:, 0], in1=bzv,
                            op=mybir.AluOpType.add,
                        )
                    else:
                        nc.vector.tensor_tensor(
                            out=x1[:], in0=pB[:], in1=bzv,
                            op=mybir.AluOpType.add,
                        )
                    del idx_tiles[ci]
                    # softplus(x) - ln2 == Ln(0.5*Exp(x) + 0.5)
                    ex = p2o.tile([128, 8, OUT_F], F32, tag="ex")
                    nc.scalar.activation(
                        out=ex[:], in_=x1[:],
                        func=mybir.ActivationFunctionType.Exp,
                    )
                    x2 = p2o.tile([128, 8, OUT_F], F32, tag="x2")
                    nc.scalar.activation(
                        out=x2[:], in_=ex[:],
                        func=mybir.ActivationFunctionType.Ln,
                        scale=0.5,
                        bias=half_sb[:],
                    )
                    # staging write: row r = 1024*c + col*128 + p
                    nc.sync.dma_start(
                        out=stag[g][1024 * c : 1024 * (c + 1), :].rearrange(
                            "(a p) f -> p a f", p=128
                        ),
                        in_=x2[:],
                    )

                uidx_tiles = {}

                def load_uidx(g, q, j):
                    it = p3i.tile([128, IWRAP], I16, tag="ui")
                    nc.sync.dma_start(
                        out=it[:], in_=uidx[g, q - PERMS_PER_G * g, j]
                    )
                    uidx_tiles[(g, q, j)] = it

                def unsort(g, q, j, first):
                    """Gather-back perm q (global), site entries 1024j..+1024."""
                    it = uidx_tiles.pop((g, q, j))
                    ut = p3g.tile([128, 8, OUT_F], F32, tag="ut")
                    dma_gather_128(
                        nc,
                        out_ap=ut[:],
                        in_ap=stag[g][:, 0:64],
                        idxs_ap=it[:, :],
                        num_idxs=NIDX,
                        elem_size=64,
                        elem_step=64,
                        queue_num=(q + j) % 4,
                    )
                    if first:
                        nc.vector.tensor_copy(out=acc[j][:], in_=ut[:])
                    else:
                        nc.vector.tensor_tensor(
                            out=acc[j][:], in0=acc[j][:], in1=ut[:],
                            op=mybir.AluOpType.add,
                        )

                # issue: B-stream D_AHEAD chunks ahead of the A-stream;
                # unsort work for group g is woven into group g+1's chunks
                unsort_work = []  # closures pending
                for ci in range(min(D_AHEAD + L_AHEAD, nchunks)):
                    load_idx(ci)
                for ci in range(min(D_AHEAD, nchunks)):
                    b_calls(ci)
                for ci in range(nchunks):
                    a_calls_and_finish(ci)
                    if ci + D_AHEAD < nchunks:
                        b_calls(ci + D_AHEAD)
                    if ci + D_AHEAD + L_AHEAD < nchunks:
                        load_idx(ci + D_AHEAD + L_AHEAD)
                    g, c, _, _, _ = chunks[ci]
                    if c == CH_PER_G - 1:
                        # group g staging complete -> queue its unsort work
                        for qq in range(PERMS_PER_G):
                            q = PERMS_PER_G * g + qq
                            load_uidx(g, q, 0) if qq == 0 else None
                            for j in range(U_CALLS_PER_PERM):
                                unsort_work.append((g, q, j, q == 0))
                    # interleave pending unsort work (2 per chunk)
                    for _ in range(2):
                        if unsort_work:
                            w = unsort_work.pop(0)
                            # prefetch the idx for the call after next
                            ahead = unsort_work[3] if len(unsort_work) > 3 else None
                            if w[:3] not in uidx_tiles:
                                load_uidx(*w[:3])
                            unsort(*w)
                            if ahead is not None and ahead[:3] not in uidx_tiles:
                                load_uidx(*ahead[:3])
                while unsort_work:
                    w = unsort_work.pop(0)
                    if w[:3] not in uidx_tiles:
                        load_uidx(*w[:3])
                    unsort(*w)

                for j in range(U_CALLS_PER_PERM):
                    nc.sync.dma_start(out=out[j], in_=acc[j][:])

    nc.compile()
    return nc


# ---------------------------------------------------------------- host side
def _gray8(isB):
    k = (isB * (1 << np.arange(8, dtype=np.int64))[::-1]).sum(1)
    b = k.copy()
    s = 1
    while s < 8:
        b ^= b >> s
        s *= 2
    return b


def _wrap16(sites):
    """[ncall, 1024] int16 -> [ncall, 128, IWRAP] (16-wrap, 8 replicas)."""
    ncall = sites.shape[0]
    wrap = np.zeros((ncall, 16, IWRAP), dtype=np.int16)
    ar = np.arange(NIDX)
    wrap[:, ar % 16, ar // 16] = sites
    return np.tile(wrap, (1, 8, 1))


def _host_prep(X_sites, X_NSs, W, b):
    X_sites = np.asarray(X_sites, dtype=np.float32)
    X_NSs = np.asarray(X_NSs)
    W = np.asarray(W, dtype=np.float32)
    b = np.asarray(b, dtype=np.float32)

    xt = np.zeros((128, XT_HALF), dtype=np.float32)
    xt[:64, :] = X_sites[:XT_HALF].T
    xt[64:, : N_SITES - XT_HALF] = X_sites[XT_HALF:].T
    xt = xt.astype(np.float16)

    wt = np.ascontiguousarray(
        np.tile(
            W.reshape(OUT_F, N_NEIGH, NODE_F).transpose(2, 1, 0).reshape(NODE_F, 512),
            (2, 1),
        )
    ).astype(np.float16)
    bz = np.ascontiguousarray(
        np.broadcast_to(np.tile(b, N_NEIGH).reshape(1, 512), (128, 512))
    ).astype(np.float16)

    # ---- per-core sorted orders and bank needs
    per_core = []
    for c in range(N_CORES):
        ns = np.asarray(
            X_NSs[c * SITES_PER_CORE : (c + 1) * SITES_PER_CORE], dtype=np.int64
        )  # [6250, 12, 8]
        groups = []
        for g in range(N_GROUPS):
            sub = ns[:, g * PERMS_PER_G : (g + 1) * PERMS_PER_G, :].reshape(
                POS_PER_G, N_NEIGH
            )
            isB = sub >= BANK
            # descending gray key: all-B patterns first, so early chunks are
            # bank-B-heavy (tabB is written first) and the A-call stream has
            # time to wait out the tabA writes
            order = np.argsort(-_gray8(isB), kind="stable")
            ssub = sub[order]                       # [25000, 8]
            pad = CAP_PER_G - POS_PER_G
            ssub = np.concatenate(
                [ssub, np.full((pad, N_NEIGH), -1, dtype=np.int64)]
            )
            groups.append((ssub, order))
        per_core.append(groups)

    # ---- shared call structure (union across cores)
    structure = []
    for g in range(N_GROUPS):
        gs = []
        for c in range(CH_PER_G):
            needA = np.zeros(N_NEIGH, bool)
            needB = np.zeros(N_NEIGH, bool)
            for core in range(N_CORES):
                ssub = per_core[core][g][0][1024 * c : 1024 * (c + 1)]
                real = ssub >= 0
                needA |= ((ssub <= BANK - 1) & real).any(0)
                needB |= (ssub >= BANK).any(0)
            aslots = tuple(np.nonzero(needA)[0].tolist())
            bslots = tuple(np.nonzero(needB)[0].tolist())
            if not aslots and not bslots:
                aslots = (0,)                        # degenerate all-pad chunk
            gs.append((aslots, bslots))
        structure.append(tuple(gs))
    structure = tuple(structure)

    # ---- per-core input tensors
    in_maps = []
    for core in range(N_CORES):
        gidx_parts = []
        for g in range(N_GROUPS):
            ssub = per_core[core][g][0]
            for c in range(CH_PER_G):
                aslots, bslots = structure[g][c]
                blockk = ssub[1024 * c : 1024 * (c + 1)]
                calls = []
                for k in bslots:
                    s = blockk[:, k]
                    calls.append(
                        np.where(s >= BANK, s - (BANK - 1), 0).astype(np.int16)
                    )
                for k in aslots:
                    s = blockk[:, k]
                    calls.append(
                        np.where((s >= 0) & (s <= BANK - 1), s + 1, 0).astype(
                            np.int16
                        )
                    )
                w = _wrap16(np.stack(calls))          # [ncalls, 128, IWRAP]
                gidx_parts.append(
                    w.transpose(1, 0, 2).reshape(128, -1)
                )
        gidx_full = np.ascontiguousarray(np.concatenate(gidx_parts, axis=1))

        # unsort indices: staging row of (site, perm)
        ui = np.zeros(
            (N_GROUPS, PERMS_PER_G, U_CALLS_PER_PERM, NIDX), dtype=np.int16
        )
        for g in range(N_GROUPS):
            order = per_core[core][g][1]
            inv = np.empty(POS_PER_G, dtype=np.int64)
            inv[order] = np.arange(POS_PER_G)
            # original position (n, qq): n*PERMS_PER_G + qq
            for qq in range(PERMS_PER_G):
                rows = inv[
                    np.arange(SITES_PER_CORE) * PERMS_PER_G + qq
                ]  # staging row per site
                rows = np.concatenate(
                    [rows, np.zeros(U_SITES - SITES_PER_CORE, dtype=np.int64)]
                )
                ui[g, qq] = rows.reshape(U_CALLS_PER_PERM, NIDX)
        uidx_full = _wrap16(
            ui.reshape(-1, NIDX)
        ).reshape(N_GROUPS, PERMS_PER_G, U_CALLS_PER_PERM, 128, IWRAP)

        in_maps.append(
            {"xt": xt, "wt": wt, "bz": bz, "gidx": gidx_full, "uidx": uidx_full}
        )
    return structure, in_maps


_NC_CACHE = {}


def _get_nc(structure=None):
    if structure is None:
        return _NC_CACHE["last"]
    if structure not in _NC_CACHE:
        _NC_CACHE[structure] = build_nc(structure)
    _NC_CACHE["last"] = _NC_CACHE[structure]
    return _NC_CACHE[structure]


def _stitch(results):
    full = np.empty((N_SITES, OUT_F), dtype=np.float32)
    for c, r in enumerate(results):
        o = (
            r["out"]                       # [7, 128, 8, 64]; site = j*1024+a*128+p
            .transpose(0, 2, 1, 3)         # [7, 8, 128, 64]
            .reshape(U_SITES, OUT_F)[:SITES_PER_CORE]
        )
        full[c * SITES_PER_CORE : (c + 1) * SITES_PER_CORE] = o
    return full


def kernel(X_sites, X_NSs, W, b, _trace=False):
    structure, in_maps = _host_prep(X_sites, X_NSs, W, b)
    nc = _get_nc(structure)
    res = run_bass_kernel_spmd(
        nc, in_maps, core_ids=list(range(N_CORES)), trace=_trace
    )
    full = _stitch(res.results)
    if _trace:
        return full, res
    return full
